# revision 19
# baseline (speedup 1.0000x reference)
"""Trainium2 Bass kernel for nn_Custom_Final_Pooling_2D (segment_reduce).

Computes out = einsum("rn,bn->br", T, x*x) where T is the fixed binary
2x2-pooling selector built by the reference's build_pooling_matrix(32, 16):
  - T has shape [496, 1024]; only rows r0(l)+c are nonzero, where
    r0(l) = 31*l - l*(l+1)//2 + 15, for l, c in [0, 16).
  - Row r0(l)+c sums x[.., i*32+j]^2 over the 2x2 window
    i in {2l, 2l+1}, j in {2c, 2c+1}.

So the kernel is: square (ScalarE, in place), pairwise add along j
(VectorE, stride-2), pairwise add along i (VectorE) into the dense
[rows, 256] pool result stored as float16, then a contiguous DMA store
of that dense layout. The host upcasts to f32, scatters the 16 column
blocks to offsets r0(l) and materializes the 240 always-zero columns
while gathering. (Writing the 496- or 361-wide layouts on device moves
27-48% more bytes; strided partial-width stores measured ~1.9x slower
per byte than contiguous. f16 halves the store bytes again; the rel-err
cost is ~1e-4 against a 2e-2 budget.)

The chunk schedule tail-splits the final supertile (8,8,...,8,4,2,1,1
rows-per-partition) so the last load's dependent compute+store chain is
~1/8 the length: with HBM bandwidth saturated by the load stream, the
end-of-pass compute tail is the only exposed latency.

Data-parallel over 8 NeuronCores: batch dim sharded 65536 -> 8 x 8192.
"""

import numpy as np

import concourse.bacc as bacc
import concourse.mybir as mybir
from concourse.tile import TileContext
from concourse.bass_utils import run_bass_kernel_spmd

N_CORES = 8
BATCH = 65536
IMG = 32          # input image side
OUT_SIDE = 16     # pooled side
N_FEAT = IMG * IMG          # 1024
N_OUT = (2 * OUT_SIDE) * (2 * OUT_SIDE - 1) // 2  # 496
ROWS_PER_CORE = BATCH // N_CORES  # 8192

P = 128           # SBUF partitions
R = 8             # batch rows per partition per supertile
SUPER = P * R     # 1024 batch rows per supertile
N_TILES = ROWS_PER_CORE // SUPER  # 8

# Nonzero-row offsets of T: line l's 16 outputs live at columns
# r0(l) .. r0(l)+15 of the 496-wide output. Cols < 15 and >= 376 are
# always zero (as are the interior gaps); they stay at the memset value.
R0 = [31 * l - l * (l + 1) // 2 + 15 for l in range(OUT_SIDE)]

# The device writes the fully dense [rows, 256] pool output (line-major
# 16x16 blocks) — the exact nonzero values, contiguous, at full write
# bandwidth with 48% fewer bytes than the 496-wide layout; the host
# scatters the 16 column blocks to offsets R0[l] during the gather step.
N_ACT = OUT_SIDE * OUT_SIDE           # 256

# Output columns outside [OUT_LO, OUT_HI) are always zero.
OUT_LO = R0[0]                        # 15
OUT_HI = R0[OUT_SIDE - 1] + OUT_SIDE  # 376

_CACHE = {}


def build_program(rows: int = ROWS_PER_CORE, r: int = R, repeat: int = 1,
                  internal_io: bool = False, mode: str = "full"):
    """Build the per-core Bass program: x [rows, 1024] -> out [rows, 496].

    repeat > 1 wraps the whole body in a hardware For_i loop that redoes
    the identical pass `repeat` times — used only for benchmarking (the
    slope over `repeat` isolates on-device time from host overhead).

    internal_io=True replaces the I/O tensors with internal DRAM buffers
    (plus a dummy [1,1] external output) so benchmark calls skip the
    256 MiB host<->device transfer entirely. The instruction stream is
    identical to the real program.

    mode: "full" (real kernel) | "in_only" | "out_only" | "inout"
    (loads + dep-free stores on separate queues, overlap allowed) |
    "inout_ser" (loads then stores on one queue, strictly serialized)
    — DMA stream probes for benchmarking.
    """
    nc = bacc.Bacc("TRN2", target_bir_lowering=False, debug=False,
                   num_devices=N_CORES)
    f32 = mybir.dt.float32
    f16 = mybir.dt.float16
    if internal_io:
        x = nc.dram_tensor("xbuf", [rows, N_FEAT], f32).ap()
        out = nc.dram_tensor("obuf", [rows, N_ACT], f16).ap()
        dummy = nc.dram_tensor("out", [1, 1], f32, kind="ExternalOutput").ap()
    else:
        x = nc.dram_tensor("x", [rows, N_FEAT], f32,
                           kind="ExternalInput").ap()
        out = nc.dram_tensor("out", [rows, N_ACT], f16,
                             kind="ExternalOutput").ap()

    # Chunk schedule: uniform r-row supertiles. (Tail-split variants
    # measured SLOWER on the load stream — 98.4 vs 95.7 us — and the
    # deferred-store schedule below removes the compute-tail motivation.)
    r_units = rows // P
    chunk_rs = [r] * (r_units // r)
    rest = r_units - sum(chunk_rs)
    if rest:
        chunk_rs.append(rest)
    assert sum(chunk_rs) == r_units

    # Per chunk: partition p holds rr consecutive batch rows.
    def x_view(row0, rr):
        return x[row0:row0 + P * rr].rearrange("(p r) m -> p (r m)",
                                               p=P, r=rr)

    def o_view(row0, rr):
        return out[row0:row0 + P * rr].rearrange("(p r) m -> p (r m)",
                                                 p=P, r=rr)

    with TileContext(nc) as tc:
        with tc.tile_pool(name="xin", bufs=3) as xin_pool, \
             tc.tile_pool(name="y1", bufs=2) as y1_pool, \
             tc.tile_pool(name="y2", bufs=1) as y2_pool:
            if internal_io:
                # zero-fill the internal input region once so the bench
                # never squares NaN/Inf garbage, and feed the dummy output
                zt = xin_pool.tile([P, r * N_FEAT], f32, tag="xt")
                nc.gpsimd.memset(zt[:], 0.0)
                ztb = y2_pool.tile([P, r * N_ACT], f16, tag="y2z")
                nc.gpsimd.memset(ztb[:], 0.0)
                row0 = 0
                for rr in chunk_rs:
                    nc.sync.dma_start(out=x_view(row0, rr),
                                      in_=zt[:, :rr * N_FEAT])
                    row0 += P * rr
                nc.sync.dma_start(out=dummy, in_=zt[:1, :1])

            def body():
                row0 = 0
                if mode in ("inout", "inout_ser"):
                    st_engine = nc.gpsimd if mode == "inout" else nc.sync
                    for rr in chunk_rs:
                        xt = xin_pool.tile([P, rr * N_FEAT], f32, tag="xt")
                        nc.sync.dma_start(out=xt[:], in_=x_view(row0, rr))
                        row0 += P * rr
                    row0 = 0
                    for rr in chunk_rs:
                        st_engine.dma_start(out=o_view(row0, rr),
                                            in_=ztb[:, :rr * N_ACT])
                        row0 += P * rr
                    return
                if mode == "in2":
                    for t, rr in enumerate(chunk_rs):
                        xt = xin_pool.tile([P, rr * N_FEAT], f32, tag="xt")
                        eng = nc.sync if t % 2 == 0 else nc.scalar
                        eng.dma_start(out=xt[:], in_=x_view(row0, rr))
                        row0 += P * rr
                    return
                if mode == "out2":
                    for t, rr in enumerate(chunk_rs):
                        eng = nc.sync if t % 2 == 0 else nc.scalar
                        eng.dma_start(out=o_view(row0, rr),
                                      in_=ztb[:, :rr * N_ACT])
                        row0 += P * rr
                    return
                # Chunks 0..n-3 share one y2 tile so their stores merge
                # into a single DMA (per-DMA setup gaps measured ~1.3 us
                # across an 8-DMA store stream). The last two chunks get
                # their own tiles/stores so the merged store never waits
                # on the end-of-pass compute.
                n_ch = len(chunk_rs)
                merged_n = n_ch - 2 if (
                    n_ch > 2 and all(c == r for c in chunk_rs[:n_ch - 2])
                    and mode == "full") else 0
                if merged_n:
                    y2a = y2_pool.tile([P, merged_n * r * N_ACT], f16,
                                       tag="y2a")
                stores = []
                for t, rr in enumerate(chunk_rs):
                    if mode == "out_only":
                        nc.scalar.dma_start(out=o_view(row0, rr),
                                            in_=ztb[:, :rr * N_ACT])
                        row0 += P * rr
                        continue
                    xt = xin_pool.tile([P, rr * N_FEAT], f32, tag="xt")
                    nc.sync.dma_start(out=xt[:], in_=x_view(row0, rr))
                    if mode == "in_only":
                        row0 += P * rr
                        continue

                    # square in place (elementwise, same AP — safe)
                    nc.scalar.activation(xt[:], xt[:],
                                         mybir.ActivationFunctionType.Square)

                    # pool over j: y1[p, 512rr], index = 512*row + 16*i + c
                    y1 = y1_pool.tile([P, rr * N_FEAT // 2], f32, tag="y1")
                    nc.vector.tensor_add(y1[:], xt[:, 0::2], xt[:, 1::2])

                    # pool over i: one dense add into y2 [p, rr*256]
                    # (y1 viewed [p, row, l, two, c]; y2 = even + odd i)
                    y1v = y1[:].rearrange("p (row l two c) -> p row l two c",
                                          row=rr, l=OUT_SIDE, two=2,
                                          c=OUT_SIDE)
                    if t < merged_n:
                        y2 = y2a[:, t * r * N_ACT:(t + 1) * r * N_ACT]
                    else:
                        y2t = y2_pool.tile([P, rr * N_ACT], f16,
                                           tag=f"y2_{t}", name=f"y2_{t}")
                        y2 = y2t[:]
                        stores.append((row0, rr, y2))
                    y2v = y2.rearrange("p (row l c) -> p row l c",
                                       row=rr, l=OUT_SIDE, c=OUT_SIDE)
                    nc.vector.tensor_add(y2v, y1v[:, :, :, 0, :],
                                         y1v[:, :, :, 1, :])
                    row0 += P * rr

                # All stores issued on the SYNC queue AFTER every load:
                # FIFO order serializes writes behind the read stream.
                # Measured: overlapped mixed read/write traffic runs at
                # 327 GB/s, strictly serialized streams at 351 GB/s —
                # read/write turnaround on HBM makes overlap a net loss.
                # By the time the loads drain, every chunk's compute but
                # the last is done, so the stores stream back-to-back.
                if mode == "full" and merged_n:
                    nc.sync.dma_start(
                        out=out[0:merged_n * P * r].rearrange(
                            "(c p r) m -> p c r m", c=merged_n, p=P, r=r),
                        in_=y2a[:].rearrange("p (c r m) -> p c r m",
                                             c=merged_n, r=r, m=N_ACT))
                for row0_s, rr, y2 in stores:
                    nc.sync.dma_start(out=o_view(row0_s, rr), in_=y2)

            if repeat == 1:
                body()
            else:
                with tc.For_i(0, repeat, 1):
                    body()

    nc.compile()
    return nc


def kernel(**inputs) -> np.ndarray:
    x = np.ascontiguousarray(inputs["input_state"], dtype=np.float32)
    assert x.shape == (BATCH, N_FEAT), x.shape

    if "nc" not in _CACHE:
        _CACHE["nc"] = build_program()
    nc = _CACHE["nc"]

    shards = [x[i * ROWS_PER_CORE:(i + 1) * ROWS_PER_CORE]
              for i in range(N_CORES)]
    in_maps = [{"x": s} for s in shards]
    res = run_bass_kernel_spmd(nc, in_maps, list(range(N_CORES)))

    # gather + unshard: upcast the f16 device output, scatter the dense
    # 16-col blocks to R0[l] and materialize the always-zero columns
    compact = np.concatenate([np.asarray(res.results[i]["out"])
                              for i in range(N_CORES)], axis=0)
    compact = compact.astype(np.float32)
    full = np.zeros((BATCH, N_OUT), dtype=np.float32)
    for l in range(OUT_SIDE):
        full[:, R0[l]:R0[l] + OUT_SIDE] = \
            compact[:, l * OUT_SIDE:(l + 1) * OUT_SIDE]
    return full



# revision 20
# speedup vs baseline: 1.0160x; 1.0160x over previous
"""Trainium2 Bass kernel for nn_Custom_Final_Pooling_2D (segment_reduce).

Computes out = einsum("rn,bn->br", T, x*x) where T is the fixed binary
2x2-pooling selector built by the reference's build_pooling_matrix(32, 16):
  - T has shape [496, 1024]; only rows r0(l)+c are nonzero, where
    r0(l) = 31*l - l*(l+1)//2 + 15, for l, c in [0, 16).
  - Row r0(l)+c sums x[.., i*32+j]^2 over the 2x2 window
    i in {2l, 2l+1}, j in {2c, 2c+1}.

So the kernel is: square (ScalarE, in place), pairwise add along j
(VectorE, stride-2), pairwise add along i (VectorE) into the dense
[rows, 256] pool result stored as float16, then a contiguous DMA store
of that dense layout. The host upcasts to f32, scatters the 16 column
blocks to offsets r0(l) and materializes the 240 always-zero columns
while gathering. (Writing the 496- or 361-wide layouts on device moves
27-48% more bytes; strided partial-width stores measured ~1.9x slower
per byte than contiguous. f16 halves the store bytes again; the rel-err
cost is ~1e-4 against a 2e-2 budget.)

The chunk schedule tail-splits the final supertile (8,8,...,8,4,2,1,1
rows-per-partition) so the last load's dependent compute+store chain is
~1/8 the length: with HBM bandwidth saturated by the load stream, the
end-of-pass compute tail is the only exposed latency.

Data-parallel over 8 NeuronCores: batch dim sharded 65536 -> 8 x 8192.
"""

import numpy as np

import concourse.bacc as bacc
import concourse.mybir as mybir
from concourse.tile import TileContext
from concourse.bass_utils import run_bass_kernel_spmd

N_CORES = 8
BATCH = 65536
IMG = 32          # input image side
OUT_SIDE = 16     # pooled side
N_FEAT = IMG * IMG          # 1024
N_OUT = (2 * OUT_SIDE) * (2 * OUT_SIDE - 1) // 2  # 496
ROWS_PER_CORE = BATCH // N_CORES  # 8192

P = 128           # SBUF partitions
R = 8             # batch rows per partition per supertile
SUPER = P * R     # 1024 batch rows per supertile
N_TILES = ROWS_PER_CORE // SUPER  # 8

# Nonzero-row offsets of T: line l's 16 outputs live at columns
# r0(l) .. r0(l)+15 of the 496-wide output. Cols < 15 and >= 376 are
# always zero (as are the interior gaps); they stay at the memset value.
R0 = [31 * l - l * (l + 1) // 2 + 15 for l in range(OUT_SIDE)]

# The device writes the fully dense [rows, 256] pool output (line-major
# 16x16 blocks) — the exact nonzero values, contiguous, at full write
# bandwidth with 48% fewer bytes than the 496-wide layout; the host
# scatters the 16 column blocks to offsets R0[l] during the gather step.
N_ACT = OUT_SIDE * OUT_SIDE           # 256

# Output columns outside [OUT_LO, OUT_HI) are always zero.
OUT_LO = R0[0]                        # 15
OUT_HI = R0[OUT_SIDE - 1] + OUT_SIDE  # 376

_CACHE = {}


def build_program(rows: int = ROWS_PER_CORE, r: int = R, repeat: int = 1,
                  internal_io: bool = False, mode: str = "full"):
    """Build the per-core Bass program: x [rows, 1024] -> out [rows, 496].

    repeat > 1 wraps the whole body in a hardware For_i loop that redoes
    the identical pass `repeat` times — used only for benchmarking (the
    slope over `repeat` isolates on-device time from host overhead).

    internal_io=True replaces the I/O tensors with internal DRAM buffers
    (plus a dummy [1,1] external output) so benchmark calls skip the
    256 MiB host<->device transfer entirely. The instruction stream is
    identical to the real program.

    mode: "full" (real kernel) | "in_only" | "out_only" | "inout"
    (loads + dep-free stores on separate queues, overlap allowed) |
    "inout_ser" (loads then stores on one queue, strictly serialized)
    — DMA stream probes for benchmarking.
    """
    nc = bacc.Bacc("TRN2", target_bir_lowering=False, debug=False,
                   num_devices=N_CORES)
    f32 = mybir.dt.float32
    f16 = mybir.dt.float16
    if internal_io:
        x = nc.dram_tensor("xbuf", [rows, N_FEAT], f32).ap()
        out = nc.dram_tensor("obuf", [rows, N_ACT], f16).ap()
        dummy = nc.dram_tensor("out", [1, 1], f32, kind="ExternalOutput").ap()
    else:
        x = nc.dram_tensor("x", [rows, N_FEAT], f32,
                           kind="ExternalInput").ap()
        out = nc.dram_tensor("out", [rows, N_ACT], f16,
                             kind="ExternalOutput").ap()

    # Chunk schedule: uniform r-row supertiles. (Tail-split variants
    # measured SLOWER on the load stream — 98.4 vs 95.7 us — and the
    # deferred-store schedule below removes the compute-tail motivation.)
    r_units = rows // P
    chunk_rs = [r] * (r_units // r)
    rest = r_units - sum(chunk_rs)
    if rest:
        chunk_rs.append(rest)
    assert sum(chunk_rs) == r_units

    # Per chunk: partition p holds rr consecutive batch rows.
    def x_view(row0, rr):
        return x[row0:row0 + P * rr].rearrange("(p r) m -> p (r m)",
                                               p=P, r=rr)

    def o_view(row0, rr):
        return out[row0:row0 + P * rr].rearrange("(p r) m -> p (r m)",
                                                 p=P, r=rr)

    with TileContext(nc) as tc:
        with tc.tile_pool(name="xin", bufs=3) as xin_pool, \
             tc.tile_pool(name="y1", bufs=2) as y1_pool, \
             tc.tile_pool(name="y2", bufs=1) as y2_pool:
            if internal_io:
                # zero-fill the internal input region once so the bench
                # never squares NaN/Inf garbage, and feed the dummy output
                zt = xin_pool.tile([P, r * N_FEAT], f32, tag="xt")
                nc.gpsimd.memset(zt[:], 0.0)
                ztb = y2_pool.tile([P, r * N_ACT], f16, tag="y2z")
                nc.gpsimd.memset(ztb[:], 0.0)
                row0 = 0
                for rr in chunk_rs:
                    nc.sync.dma_start(out=x_view(row0, rr),
                                      in_=zt[:, :rr * N_FEAT])
                    row0 += P * rr
                nc.sync.dma_start(out=dummy, in_=zt[:1, :1])

            def body():
                row0 = 0
                if mode in ("inout", "inout_ser"):
                    st_engine = nc.gpsimd if mode == "inout" else nc.sync
                    for rr in chunk_rs:
                        xt = xin_pool.tile([P, rr * N_FEAT], f32, tag="xt")
                        nc.sync.dma_start(out=xt[:], in_=x_view(row0, rr))
                        row0 += P * rr
                    row0 = 0
                    for rr in chunk_rs:
                        st_engine.dma_start(out=o_view(row0, rr),
                                            in_=ztb[:, :rr * N_ACT])
                        row0 += P * rr
                    return
                if mode == "in2":
                    for t, rr in enumerate(chunk_rs):
                        xt = xin_pool.tile([P, rr * N_FEAT], f32, tag="xt")
                        eng = nc.sync if t % 2 == 0 else nc.scalar
                        eng.dma_start(out=xt[:], in_=x_view(row0, rr))
                        row0 += P * rr
                    return
                if mode == "out2":
                    for t, rr in enumerate(chunk_rs):
                        eng = nc.sync if t % 2 == 0 else nc.scalar
                        eng.dma_start(out=o_view(row0, rr),
                                      in_=ztb[:, :rr * N_ACT])
                        row0 += P * rr
                    return
                # Chunks 0..n-3 share one y2 tile so their stores merge
                # into a single DMA (per-DMA setup gaps measured ~1.3 us
                # across an 8-DMA store stream). The last two chunks get
                # their own tiles/stores so the merged store never waits
                # on the end-of-pass compute.
                n_ch = len(chunk_rs)
                merged_n = 0  # merged strided store measured 13 us SLOWER
                if merged_n:
                    y2a = y2_pool.tile([P, merged_n * r * N_ACT], f16,
                                       tag="y2a")
                stores = []
                for t, rr in enumerate(chunk_rs):
                    if mode == "out_only":
                        nc.scalar.dma_start(out=o_view(row0, rr),
                                            in_=ztb[:, :rr * N_ACT])
                        row0 += P * rr
                        continue
                    xt = xin_pool.tile([P, rr * N_FEAT], f32, tag="xt")
                    nc.sync.dma_start(out=xt[:], in_=x_view(row0, rr))
                    if mode == "in_only":
                        row0 += P * rr
                        continue

                    # square in place (elementwise, same AP — safe)
                    nc.scalar.activation(xt[:], xt[:],
                                         mybir.ActivationFunctionType.Square)

                    # pool over j: y1[p, 512rr], index = 512*row + 16*i + c
                    y1 = y1_pool.tile([P, rr * N_FEAT // 2], f32, tag="y1")
                    nc.vector.tensor_add(y1[:], xt[:, 0::2], xt[:, 1::2])

                    # pool over i: one dense add into y2 [p, rr*256]
                    # (y1 viewed [p, row, l, two, c]; y2 = even + odd i)
                    y1v = y1[:].rearrange("p (row l two c) -> p row l two c",
                                          row=rr, l=OUT_SIDE, two=2,
                                          c=OUT_SIDE)
                    if t < merged_n:
                        y2 = y2a[:, t * r * N_ACT:(t + 1) * r * N_ACT]
                    else:
                        y2t = y2_pool.tile([P, rr * N_ACT], f16,
                                           tag=f"y2_{t}", name=f"y2_{t}")
                        y2 = y2t[:]
                        stores.append((row0, rr, y2))
                    y2v = y2.rearrange("p (row l c) -> p row l c",
                                       row=rr, l=OUT_SIDE, c=OUT_SIDE)
                    nc.vector.tensor_add(y2v, y1v[:, :, :, 0, :],
                                         y1v[:, :, :, 1, :])
                    row0 += P * rr

                # All stores issued on the SYNC queue AFTER every load:
                # FIFO order serializes writes behind the read stream.
                # Measured: overlapped mixed read/write traffic runs at
                # 327 GB/s, strictly serialized streams at 351 GB/s —
                # read/write turnaround on HBM makes overlap a net loss.
                # By the time the loads drain, every chunk's compute but
                # the last is done, so the stores stream back-to-back.
                if mode == "full" and merged_n:
                    nc.sync.dma_start(
                        out=out[0:merged_n * P * r].rearrange(
                            "(c p r) m -> p c r m", c=merged_n, p=P, r=r),
                        in_=y2a[:].rearrange("p (c r m) -> p c r m",
                                             c=merged_n, r=r, m=N_ACT))
                for row0_s, rr, y2 in stores:
                    nc.sync.dma_start(out=o_view(row0_s, rr), in_=y2)

            if repeat == 1:
                body()
            else:
                with tc.For_i(0, repeat, 1):
                    body()

    nc.compile()
    return nc


def kernel(**inputs) -> np.ndarray:
    x = np.ascontiguousarray(inputs["input_state"], dtype=np.float32)
    assert x.shape == (BATCH, N_FEAT), x.shape

    if "nc" not in _CACHE:
        _CACHE["nc"] = build_program()
    nc = _CACHE["nc"]

    shards = [x[i * ROWS_PER_CORE:(i + 1) * ROWS_PER_CORE]
              for i in range(N_CORES)]
    in_maps = [{"x": s} for s in shards]
    res = run_bass_kernel_spmd(nc, in_maps, list(range(N_CORES)))

    # gather + unshard: upcast the f16 device output, scatter the dense
    # 16-col blocks to R0[l] and materialize the always-zero columns
    compact = np.concatenate([np.asarray(res.results[i]["out"])
                              for i in range(N_CORES)], axis=0)
    compact = compact.astype(np.float32)
    full = np.zeros((BATCH, N_OUT), dtype=np.float32)
    for l in range(OUT_SIDE):
        full[:, R0[l]:R0[l] + OUT_SIDE] = \
            compact[:, l * OUT_SIDE:(l + 1) * OUT_SIDE]
    return full



# revision 24
# speedup vs baseline: 1.0398x; 1.0235x over previous
"""Trainium2 Bass kernel for nn_Custom_Final_Pooling_2D (segment_reduce).

Computes out = einsum("rn,bn->br", T, x*x) where T is the fixed binary
2x2-pooling selector built by the reference's build_pooling_matrix(32, 16):
  - T has shape [496, 1024]; only rows r0(l)+c are nonzero, where
    r0(l) = 31*l - l*(l+1)//2 + 15, for l, c in [0, 16).
  - Row r0(l)+c sums x[.., i*32+j]^2 over the 2x2 window
    i in {2l, 2l+1}, j in {2c, 2c+1}.

So the kernel is: square (ScalarE, in place), pairwise add along j
(VectorE, stride-2), pairwise add along i (VectorE) into the dense
[rows, 256] pool result stored as float16, then a contiguous DMA store
of that dense layout. The host upcasts to f32, scatters the 16 column
blocks to offsets r0(l) and materializes the 240 always-zero columns
while gathering. (Writing the 496- or 361-wide layouts on device moves
27-48% more bytes; strided partial-width stores measured ~1.9x slower
per byte than contiguous. f16 halves the store bytes again; the rel-err
cost is ~1e-4 against a 2e-2 budget.)

The chunk schedule tail-splits the final supertile (8,8,...,8,4,2,1,1
rows-per-partition) so the last load's dependent compute+store chain is
~1/8 the length: with HBM bandwidth saturated by the load stream, the
end-of-pass compute tail is the only exposed latency.

Data-parallel over 8 NeuronCores: batch dim sharded 65536 -> 8 x 8192.
"""

import numpy as np

import concourse.bacc as bacc
import concourse.mybir as mybir
from concourse.tile import TileContext
from concourse.bass_utils import run_bass_kernel_spmd

N_CORES = 8
BATCH = 65536
IMG = 32          # input image side
OUT_SIDE = 16     # pooled side
N_FEAT = IMG * IMG          # 1024
N_OUT = (2 * OUT_SIDE) * (2 * OUT_SIDE - 1) // 2  # 496
ROWS_PER_CORE = BATCH // N_CORES  # 8192

P = 128           # SBUF partitions
R = 8             # batch rows per partition per supertile
SUPER = P * R     # 1024 batch rows per supertile
N_TILES = ROWS_PER_CORE // SUPER  # 8

# Nonzero-row offsets of T: line l's 16 outputs live at columns
# r0(l) .. r0(l)+15 of the 496-wide output. Cols < 15 and >= 376 are
# always zero (as are the interior gaps); they stay at the memset value.
R0 = [31 * l - l * (l + 1) // 2 + 15 for l in range(OUT_SIDE)]

# The device writes the fully dense [rows, 256] pool output (line-major
# 16x16 blocks) — the exact nonzero values, contiguous, at full write
# bandwidth with 48% fewer bytes than the 496-wide layout; the host
# scatters the 16 column blocks to offsets R0[l] during the gather step.
N_ACT = OUT_SIDE * OUT_SIDE           # 256

# Output columns outside [OUT_LO, OUT_HI) are always zero.
OUT_LO = R0[0]                        # 15
OUT_HI = R0[OUT_SIDE - 1] + OUT_SIDE  # 376

_CACHE = {}


def build_program(rows: int = ROWS_PER_CORE, r: int = R, repeat: int = 1,
                  internal_io: bool = False, mode: str = "full",
                  split_last: bool = False, store2: bool = True):
    """Build the per-core Bass program: x [rows, 1024] -> out [rows, 496].

    repeat > 1 wraps the whole body in a hardware For_i loop that redoes
    the identical pass `repeat` times — used only for benchmarking (the
    slope over `repeat` isolates on-device time from host overhead).

    internal_io=True replaces the I/O tensors with internal DRAM buffers
    (plus a dummy [1,1] external output) so benchmark calls skip the
    256 MiB host<->device transfer entirely. The instruction stream is
    identical to the real program.

    mode: "full" (real kernel) | "in_only" | "out_only" | "inout"
    (loads + dep-free stores on separate queues, overlap allowed) |
    "inout_ser" (loads then stores on one queue, strictly serialized)
    — DMA stream probes for benchmarking.
    """
    nc = bacc.Bacc("TRN2", target_bir_lowering=False, debug=False,
                   num_devices=N_CORES)
    f32 = mybir.dt.float32
    f16 = mybir.dt.float16
    if internal_io:
        x = nc.dram_tensor("xbuf", [rows, N_FEAT], f32).ap()
        out = nc.dram_tensor("obuf", [rows, N_ACT], f16).ap()
        dummy = nc.dram_tensor("out", [1, 1], f32, kind="ExternalOutput").ap()
    else:
        x = nc.dram_tensor("x", [rows, N_FEAT], f32,
                           kind="ExternalInput").ap()
        out = nc.dram_tensor("out", [rows, N_ACT], f16,
                             kind="ExternalOutput").ap()

    # Chunk schedule: uniform r-row supertiles. (Tail-split variants
    # measured SLOWER on the load stream — 98.4 vs 95.7 us — and the
    # deferred-store schedule below removes the compute-tail motivation.)
    r_units = rows // P
    chunk_rs = [r] * (r_units // r)
    rest = r_units - sum(chunk_rs)
    if rest:
        chunk_rs.append(rest)
    assert sum(chunk_rs) == r_units

    # Per chunk: partition p holds rr consecutive batch rows.
    def x_view(row0, rr):
        return x[row0:row0 + P * rr].rearrange("(p r) m -> p (r m)",
                                               p=P, r=rr)

    def o_view(row0, rr):
        return out[row0:row0 + P * rr].rearrange("(p r) m -> p (r m)",
                                                 p=P, r=rr)

    with TileContext(nc) as tc:
        with tc.tile_pool(name="xin", bufs=3) as xin_pool, \
             tc.tile_pool(name="y1", bufs=2) as y1_pool, \
             tc.tile_pool(name="y2", bufs=1) as y2_pool:
            if internal_io:
                # zero-fill the internal input region once so the bench
                # never squares NaN/Inf garbage, and feed the dummy output
                zt = xin_pool.tile([P, r * N_FEAT], f32, tag="xt")
                nc.gpsimd.memset(zt[:], 0.0)
                ztb = y2_pool.tile([P, r * N_ACT], f16, tag="y2z")
                nc.gpsimd.memset(ztb[:], 0.0)
                row0 = 0
                for rr in chunk_rs:
                    nc.sync.dma_start(out=x_view(row0, rr),
                                      in_=zt[:, :rr * N_FEAT])
                    row0 += P * rr
                nc.sync.dma_start(out=dummy, in_=zt[:1, :1])

            def body():
                row0 = 0
                if mode in ("inout", "inout_ser"):
                    st_engine = nc.gpsimd if mode == "inout" else nc.sync
                    for rr in chunk_rs:
                        xt = xin_pool.tile([P, rr * N_FEAT], f32, tag="xt")
                        nc.sync.dma_start(out=xt[:], in_=x_view(row0, rr))
                        row0 += P * rr
                    row0 = 0
                    for rr in chunk_rs:
                        st_engine.dma_start(out=o_view(row0, rr),
                                            in_=ztb[:, :rr * N_ACT])
                        row0 += P * rr
                    return
                if mode == "in2":
                    for t, rr in enumerate(chunk_rs):
                        xt = xin_pool.tile([P, rr * N_FEAT], f32, tag="xt")
                        eng = nc.sync if t % 2 == 0 else nc.scalar
                        eng.dma_start(out=xt[:], in_=x_view(row0, rr))
                        row0 += P * rr
                    return
                if mode == "out2":
                    for t, rr in enumerate(chunk_rs):
                        eng = nc.sync if t % 2 == 0 else nc.scalar
                        eng.dma_start(out=o_view(row0, rr),
                                      in_=ztb[:, :rr * N_ACT])
                        row0 += P * rr
                    return
                # Chunks 0..n-3 share one y2 tile so their stores merge
                # into a single DMA (per-DMA setup gaps measured ~1.3 us
                # across an 8-DMA store stream). The last two chunks get
                # their own tiles/stores so the merged store never waits
                # on the end-of-pass compute.
                n_ch = len(chunk_rs)
                merged_n = 0  # merged strided store measured 13 us SLOWER
                if merged_n:
                    y2a = y2_pool.tile([P, merged_n * r * N_ACT], f16,
                                       tag="y2a")
                stores = []
                for t, rr in enumerate(chunk_rs):
                    if mode == "out_only":
                        nc.scalar.dma_start(out=o_view(row0, rr),
                                            in_=ztb[:, :rr * N_ACT])
                        row0 += P * rr
                        continue
                    xt = xin_pool.tile([P, rr * N_FEAT], f32, tag="xt")
                    nc.sync.dma_start(out=xt[:], in_=x_view(row0, rr))
                    if mode == "in_only":
                        row0 += P * rr
                        continue

                    # square in place (elementwise, same AP — safe). For
                    # the LAST chunk, split the square between ScalarE
                    # and VectorE: its chain runs after the final load
                    # with nothing left to hide it, so latency counts.
                    if split_last and t == len(chunk_rs) - 1 and rr > 1:
                        half = (rr // 2) * N_FEAT
                        nc.scalar.activation(
                            xt[:, :half], xt[:, :half],
                            mybir.ActivationFunctionType.Square)
                        nc.vector.tensor_mul(xt[:, half:], xt[:, half:],
                                             xt[:, half:])
                    else:
                        nc.scalar.activation(
                            xt[:], xt[:],
                            mybir.ActivationFunctionType.Square)

                    # pool over j: y1[p, 512rr], index = 512*row + 16*i + c
                    y1 = y1_pool.tile([P, rr * N_FEAT // 2], f32, tag="y1")
                    nc.vector.tensor_add(y1[:], xt[:, 0::2], xt[:, 1::2])

                    # pool over i: one dense add into y2 [p, rr*256]
                    # (y1 viewed [p, row, l, two, c]; y2 = even + odd i)
                    y1v = y1[:].rearrange("p (row l two c) -> p row l two c",
                                          row=rr, l=OUT_SIDE, two=2,
                                          c=OUT_SIDE)
                    if t < merged_n:
                        y2 = y2a[:, t * r * N_ACT:(t + 1) * r * N_ACT]
                    else:
                        y2t = y2_pool.tile([P, rr * N_ACT], f16,
                                           tag=f"y2_{t}", name=f"y2_{t}")
                        y2 = y2t[:]
                        stores.append((row0, rr, y2))
                    y2v = y2.rearrange("p (row l c) -> p row l c",
                                       row=rr, l=OUT_SIDE, c=OUT_SIDE)
                    nc.vector.tensor_add(y2v, y1v[:, :, :, 0, :],
                                         y1v[:, :, :, 1, :])
                    row0 += P * rr

                # All stores issued on the SYNC queue AFTER every load:
                # FIFO order serializes writes behind the read stream.
                # Measured: overlapped mixed read/write traffic runs at
                # 327 GB/s, strictly serialized streams at 351 GB/s —
                # read/write turnaround on HBM makes overlap a net loss.
                # By the time the loads drain, every chunk's compute but
                # the last is done, so the stores stream back-to-back.
                if mode == "full" and merged_n:
                    nc.sync.dma_start(
                        out=out[0:merged_n * P * r].rearrange(
                            "(c p r) m -> p c r m", c=merged_n, p=P, r=r),
                        in_=y2a[:].rearrange("p (c r m) -> p c r m",
                                             c=merged_n, r=r, m=N_ACT))
                # store2: alternate stores between the sync queue and the
                # scalar queue. Scalar's stores trigger only after its
                # last square (engine program order), so they never mix
                # with the load stream; two queues hide per-DMA setup
                # gaps (writes measured 347 vs 313 GB/s).
                for i, (row0_s, rr, y2) in enumerate(stores):
                    eng = nc.scalar if (store2 and i % 2 == 1) else nc.sync
                    eng.dma_start(out=o_view(row0_s, rr), in_=y2)

            if repeat == 1:
                body()
            else:
                with tc.For_i(0, repeat, 1):
                    body()

    nc.compile()
    return nc


def kernel(**inputs) -> np.ndarray:
    x = np.ascontiguousarray(inputs["input_state"], dtype=np.float32)
    assert x.shape == (BATCH, N_FEAT), x.shape

    if "nc" not in _CACHE:
        _CACHE["nc"] = build_program()
    nc = _CACHE["nc"]

    shards = [x[i * ROWS_PER_CORE:(i + 1) * ROWS_PER_CORE]
              for i in range(N_CORES)]
    in_maps = [{"x": s} for s in shards]
    res = run_bass_kernel_spmd(nc, in_maps, list(range(N_CORES)))

    # gather + unshard: upcast the f16 device output, scatter the dense
    # 16-col blocks to R0[l] and materialize the always-zero columns
    compact = np.concatenate([np.asarray(res.results[i]["out"])
                              for i in range(N_CORES)], axis=0)
    compact = compact.astype(np.float32)
    full = np.zeros((BATCH, N_OUT), dtype=np.float32)
    for l in range(OUT_SIDE):
        full[:, R0[l]:R0[l] + OUT_SIDE] = \
            compact[:, l * OUT_SIDE:(l + 1) * OUT_SIDE]
    return full



# revision 28
# speedup vs baseline: 1.0669x; 1.0261x over previous
"""Trainium2 Bass kernel for nn_Custom_Final_Pooling_2D (segment_reduce).

Computes out = einsum("rn,bn->br", T, x*x) where T is the fixed binary
2x2-pooling selector built by the reference's build_pooling_matrix(32, 16):
  - T has shape [496, 1024]; only rows r0(l)+c are nonzero, where
    r0(l) = 31*l - l*(l+1)//2 + 15, for l, c in [0, 16).
  - Row r0(l)+c sums x[.., i*32+j]^2 over the 2x2 window
    i in {2l, 2l+1}, j in {2c, 2c+1}.

So the kernel is: square (ScalarE, in place), pairwise add along j
(VectorE, stride-2), pairwise add along i (VectorE) into the dense
[rows, 256] pool result stored as float16, then contiguous DMA stores
of that dense layout. The host upcasts to f32, scatters the 16 column
blocks to offsets r0(l) and materializes the 240 always-zero columns
while gathering. (Writing the 496- or 361-wide layouts on device moves
27-48% more bytes; strided partial-width stores measured ~1.9x slower
per byte than contiguous. f16 halves the store bytes again; the rel-err
cost is ~2e-4 against a 2e-2 budget.)

Schedule: the problem is HBM-bound (per-core: 32 MiB read + 4 MiB f16
write; read stream alone measures 96-101 us at ~350 GB/s). Measured on
HW: overlapped read+write traffic runs at ~327 GB/s aggregate while
strictly serialized pure streams run at ~351 GB/s — HBM read/write
turnaround makes overlap a net loss. So every output chunk is kept
live in SBUF and ALL stores issue after ALL loads (sync-queue FIFO
provides the ordering for free); by then all chunk computes except the
last have long finished and the writes stream at full bandwidth.

Data-parallel over 8 NeuronCores: batch dim sharded 65536 -> 8 x 8192.
"""

import numpy as np

import concourse.bacc as bacc
import concourse.mybir as mybir
from concourse.tile import TileContext
from concourse.bass_utils import run_bass_kernel_spmd

N_CORES = 8
BATCH = 65536
IMG = 32          # input image side
OUT_SIDE = 16     # pooled side
N_FEAT = IMG * IMG          # 1024
N_OUT = (2 * OUT_SIDE) * (2 * OUT_SIDE - 1) // 2  # 496
ROWS_PER_CORE = BATCH // N_CORES  # 8192

P = 128           # SBUF partitions
R = 8             # batch rows per partition per supertile
SUPER = P * R     # 1024 batch rows per supertile
N_TILES = ROWS_PER_CORE // SUPER  # 8

# Nonzero-row offsets of T: line l's 16 outputs live at columns
# r0(l) .. r0(l)+15 of the 496-wide output. Cols < 15 and >= 376 are
# always zero (as are the interior gaps); they stay at the memset value.
R0 = [31 * l - l * (l + 1) // 2 + 15 for l in range(OUT_SIDE)]

# The device writes the fully dense [rows, 256] pool output (line-major
# 16x16 blocks) — the exact nonzero values, contiguous, at full write
# bandwidth with 48% fewer bytes than the 496-wide layout; the host
# scatters the 16 column blocks to offsets R0[l] during the gather step.
N_ACT = OUT_SIDE * OUT_SIDE           # 256

# Output columns outside [OUT_LO, OUT_HI) are always zero.
OUT_LO = R0[0]                        # 15
OUT_HI = R0[OUT_SIDE - 1] + OUT_SIDE  # 376

_CACHE = {}


def build_program(rows: int = ROWS_PER_CORE, r: int = R, repeat: int = 1,
                  internal_io: bool = False, mode: str = "full",
                  split_last: bool = False, store2: bool = True):
    """Build the per-core Bass program: x [rows, 1024] -> out [rows, 496].

    repeat > 1 wraps the whole body in a hardware For_i loop that redoes
    the identical pass `repeat` times — used only for benchmarking (the
    slope over `repeat` isolates on-device time from host overhead).

    internal_io=True replaces the I/O tensors with internal DRAM buffers
    (plus a dummy [1,1] external output) so benchmark calls skip the
    256 MiB host<->device transfer entirely. The instruction stream is
    identical to the real program.

    mode: "full" (real kernel) | "in_only" | "out_only" | "inout"
    (loads + dep-free stores on separate queues, overlap allowed) |
    "inout_ser" (loads then stores on one queue, strictly serialized)
    — DMA stream probes for benchmarking.
    """
    nc = bacc.Bacc("TRN2", target_bir_lowering=False, debug=False,
                   num_devices=N_CORES)
    f32 = mybir.dt.float32
    f16 = mybir.dt.float16
    if internal_io:
        x = nc.dram_tensor("xbuf", [rows, N_FEAT], f32).ap()
        out = nc.dram_tensor("obuf", [rows, N_ACT], f16).ap()
        dummy = nc.dram_tensor("out", [1, 1], f32, kind="ExternalOutput").ap()
    else:
        x = nc.dram_tensor("x", [rows, N_FEAT], f32,
                           kind="ExternalInput").ap()
        out = nc.dram_tensor("out", [rows, N_ACT], f16,
                             kind="ExternalOutput").ap()

    # Chunk schedule: uniform r-row supertiles. (Tail-split variants
    # measured SLOWER on the load stream — 98.4 vs 95.7 us — and the
    # deferred-store schedule below removes the compute-tail motivation.)
    r_units = rows // P
    chunk_rs = [r] * (r_units // r)
    rest = r_units - sum(chunk_rs)
    if rest:
        chunk_rs.append(rest)
    assert sum(chunk_rs) == r_units

    # Per chunk: partition p holds rr consecutive batch rows.
    def x_view(row0, rr):
        return x[row0:row0 + P * rr].rearrange("(p r) m -> p (r m)",
                                               p=P, r=rr)

    def o_view(row0, rr):
        return out[row0:row0 + P * rr].rearrange("(p r) m -> p (r m)",
                                                 p=P, r=rr)

    with TileContext(nc) as tc:
        with tc.tile_pool(name="xin", bufs=3) as xin_pool, \
             tc.tile_pool(name="y1", bufs=2) as y1_pool, \
             tc.tile_pool(name="y2", bufs=1) as y2_pool:
            if internal_io:
                # zero-fill the internal input region once so the bench
                # never squares NaN/Inf garbage, and feed the dummy output
                zt = xin_pool.tile([P, r * N_FEAT], f32, tag="xt")
                nc.gpsimd.memset(zt[:], 0.0)
                ztb = y2_pool.tile([P, r * N_ACT], f16, tag="y2z")
                nc.gpsimd.memset(ztb[:], 0.0)
                row0 = 0
                for rr in chunk_rs:
                    nc.sync.dma_start(out=x_view(row0, rr),
                                      in_=zt[:, :rr * N_FEAT])
                    row0 += P * rr
                nc.sync.dma_start(out=dummy, in_=zt[:1, :1])

            def body():
                row0 = 0
                if mode in ("inout", "inout_ser"):
                    st_engine = nc.gpsimd if mode == "inout" else nc.sync
                    for rr in chunk_rs:
                        xt = xin_pool.tile([P, rr * N_FEAT], f32, tag="xt")
                        nc.sync.dma_start(out=xt[:], in_=x_view(row0, rr))
                        row0 += P * rr
                    row0 = 0
                    for rr in chunk_rs:
                        st_engine.dma_start(out=o_view(row0, rr),
                                            in_=ztb[:, :rr * N_ACT])
                        row0 += P * rr
                    return
                if mode == "in2":
                    for t, rr in enumerate(chunk_rs):
                        xt = xin_pool.tile([P, rr * N_FEAT], f32, tag="xt")
                        eng = nc.sync if t % 2 == 0 else nc.scalar
                        eng.dma_start(out=xt[:], in_=x_view(row0, rr))
                        row0 += P * rr
                    return
                if mode == "out2":
                    for t, rr in enumerate(chunk_rs):
                        eng = nc.sync if t % 2 == 0 else nc.scalar
                        eng.dma_start(out=o_view(row0, rr),
                                      in_=ztb[:, :rr * N_ACT])
                        row0 += P * rr
                    return
                # (A single merged strided store DMA for chunks 0..5 was
                # tried and measured ~5 us SLOWER than per-chunk
                # contiguous stores — strided DRAM writes are penalized.)
                stores = []
                for t, rr in enumerate(chunk_rs):
                    if mode == "out_only":
                        nc.scalar.dma_start(out=o_view(row0, rr),
                                            in_=ztb[:, :rr * N_ACT])
                        row0 += P * rr
                        continue
                    xt = xin_pool.tile([P, rr * N_FEAT], f32, tag="xt")
                    nc.sync.dma_start(out=xt[:], in_=x_view(row0, rr))
                    if mode == "in_only":
                        row0 += P * rr
                        continue

                    # square in place (elementwise, same AP — safe). For
                    # the LAST chunk, split the square between ScalarE
                    # and VectorE: its chain runs after the final load
                    # with nothing left to hide it, so latency counts.
                    if split_last and t == len(chunk_rs) - 1 and rr > 1:
                        half = (rr // 2) * N_FEAT
                        nc.scalar.activation(
                            xt[:, :half], xt[:, :half],
                            mybir.ActivationFunctionType.Square)
                        nc.vector.tensor_mul(xt[:, half:], xt[:, half:],
                                             xt[:, half:])
                    else:
                        nc.scalar.activation(
                            xt[:], xt[:],
                            mybir.ActivationFunctionType.Square)

                    # pool over j: y1[p, 512rr], index = 512*row + 16*i + c
                    y1 = y1_pool.tile([P, rr * N_FEAT // 2], f32, tag="y1")
                    nc.vector.tensor_add(y1[:], xt[:, 0::2], xt[:, 1::2])

                    # pool over i: one dense add into y2 [p, rr*256]
                    # (y1 viewed [p, row, l, two, c]; y2 = even + odd i)
                    y1v = y1[:].rearrange("p (row l two c) -> p row l two c",
                                          row=rr, l=OUT_SIDE, two=2,
                                          c=OUT_SIDE)
                    y2t = y2_pool.tile([P, rr * N_ACT], f16,
                                       tag=f"y2_{t}", name=f"y2_{t}")
                    y2 = y2t[:]
                    stores.append((row0, rr, y2))
                    y2v = y2.rearrange("p (row l c) -> p row l c",
                                       row=rr, l=OUT_SIDE, c=OUT_SIDE)
                    nc.vector.tensor_add(y2v, y1v[:, :, :, 0, :],
                                         y1v[:, :, :, 1, :])
                    row0 += P * rr

                # All stores issued on the SYNC queue AFTER every load:
                # FIFO order serializes writes behind the read stream.
                # Measured: overlapped mixed read/write traffic runs at
                # 327 GB/s, strictly serialized streams at 351 GB/s —
                # read/write turnaround on HBM makes overlap a net loss.
                # By the time the loads drain, every chunk's compute but
                # the last is done, so the stores stream back-to-back.
                # store2: alternate stores between the sync queue and the
                # scalar queue. Scalar's stores trigger only after its
                # last square (engine program order), so they never mix
                # with the load stream; two queues hide per-DMA setup
                # gaps (writes measured 347 vs 313 GB/s).
                for i, (row0_s, rr, y2) in enumerate(stores):
                    eng = nc.scalar if (store2 and i % 2 == 1) else nc.sync
                    eng.dma_start(out=o_view(row0_s, rr), in_=y2)

            if repeat == 1:
                body()
            else:
                with tc.For_i(0, repeat, 1):
                    body()

    nc.compile()
    return nc


def kernel(**inputs) -> np.ndarray:
    x = np.ascontiguousarray(inputs["input_state"], dtype=np.float32)
    assert x.shape == (BATCH, N_FEAT), x.shape

    if "nc" not in _CACHE:
        _CACHE["nc"] = build_program()
    nc = _CACHE["nc"]

    shards = [x[i * ROWS_PER_CORE:(i + 1) * ROWS_PER_CORE]
              for i in range(N_CORES)]
    in_maps = [{"x": s} for s in shards]
    res = run_bass_kernel_spmd(nc, in_maps, list(range(N_CORES)))

    # gather + unshard: upcast the f16 device output, scatter the dense
    # 16-col blocks to R0[l] and materialize the always-zero columns
    compact = np.concatenate([np.asarray(res.results[i]["out"])
                              for i in range(N_CORES)], axis=0)
    compact = compact.astype(np.float32)
    full = np.zeros((BATCH, N_OUT), dtype=np.float32)
    for l in range(OUT_SIDE):
        full[:, R0[l]:R0[l] + OUT_SIDE] = \
            compact[:, l * OUT_SIDE:(l + 1) * OUT_SIDE]
    return full



# revision 31
# speedup vs baseline: 1.0753x; 1.0078x over previous
"""Trainium2 Bass kernel for nn_Custom_Final_Pooling_2D (segment_reduce).

Computes out = einsum("rn,bn->br", T, x*x) where T is the fixed binary
2x2-pooling selector built by the reference's build_pooling_matrix(32, 16):
  - T has shape [496, 1024]; only rows r0(l)+c are nonzero, where
    r0(l) = 31*l - l*(l+1)//2 + 15, for l, c in [0, 16).
  - Row r0(l)+c sums x[.., i*32+j]^2 over the 2x2 window
    i in {2l, 2l+1}, j in {2c, 2c+1}.

So the kernel is: square (ScalarE, in place), pairwise add along j
(VectorE, stride-2), pairwise add along i (VectorE) into the dense
[rows, 256] pool result stored as float16, then contiguous DMA stores
of that dense layout. The host upcasts to f32, scatters the 16 column
blocks to offsets r0(l) and materializes the 240 always-zero columns
while gathering. (Writing the 496- or 361-wide layouts on device moves
27-48% more bytes; strided partial-width stores measured ~1.9x slower
per byte than contiguous. f16 halves the store bytes again; the rel-err
cost is ~2e-4 against a 2e-2 budget.)

Schedule: the problem is HBM-bound (per-core: 32 MiB read + 4 MiB f16
write; read stream alone measures 96-101 us at ~350 GB/s). Measured on
HW: overlapped read+write traffic runs at ~327 GB/s aggregate while
strictly serialized pure streams run at ~351 GB/s — HBM read/write
turnaround makes overlap a net loss. So every output chunk is kept
live in SBUF and ALL stores issue after ALL loads (sync-queue FIFO
provides the ordering for free); by then all chunk computes except the
last have long finished and the writes stream at full bandwidth.

Data-parallel over 8 NeuronCores: batch dim sharded 65536 -> 8 x 8192.
"""

import numpy as np

import concourse.bacc as bacc
import concourse.mybir as mybir
from concourse.tile import TileContext
from concourse.bass_utils import run_bass_kernel_spmd

N_CORES = 8
BATCH = 65536
IMG = 32          # input image side
OUT_SIDE = 16     # pooled side
N_FEAT = IMG * IMG          # 1024
N_OUT = (2 * OUT_SIDE) * (2 * OUT_SIDE - 1) // 2  # 496
ROWS_PER_CORE = BATCH // N_CORES  # 8192

P = 128           # SBUF partitions
R = 8             # batch rows per partition per supertile
SUPER = P * R     # 1024 batch rows per supertile
N_TILES = ROWS_PER_CORE // SUPER  # 8

# Nonzero-row offsets of T: line l's 16 outputs live at columns
# r0(l) .. r0(l)+15 of the 496-wide output. Cols < 15 and >= 376 are
# always zero (as are the interior gaps); they stay at the memset value.
R0 = [31 * l - l * (l + 1) // 2 + 15 for l in range(OUT_SIDE)]

# The device writes the fully dense [rows, 256] pool output (line-major
# 16x16 blocks) — the exact nonzero values, contiguous, at full write
# bandwidth with 48% fewer bytes than the 496-wide layout; the host
# scatters the 16 column blocks to offsets R0[l] during the gather step.
N_ACT = OUT_SIDE * OUT_SIDE           # 256

# Output columns outside [OUT_LO, OUT_HI) are always zero.
OUT_LO = R0[0]                        # 15
OUT_HI = R0[OUT_SIDE - 1] + OUT_SIDE  # 376

_CACHE = {}


def build_program(rows: int = ROWS_PER_CORE, r: int = R, repeat: int = 1,
                  internal_io: bool = False, mode: str = "full",
                  split_last: bool = False, store2: bool = True,
                  tail44: bool = True):
    """Build the per-core Bass program: x [rows, 1024] -> out [rows, 496].

    repeat > 1 wraps the whole body in a hardware For_i loop that redoes
    the identical pass `repeat` times — used only for benchmarking (the
    slope over `repeat` isolates on-device time from host overhead).

    internal_io=True replaces the I/O tensors with internal DRAM buffers
    (plus a dummy [1,1] external output) so benchmark calls skip the
    256 MiB host<->device transfer entirely. The instruction stream is
    identical to the real program.

    mode: "full" (real kernel) | "in_only" | "out_only" | "inout"
    (loads + dep-free stores on separate queues, overlap allowed) |
    "inout_ser" (loads then stores on one queue, strictly serialized)
    — DMA stream probes for benchmarking.
    """
    nc = bacc.Bacc("TRN2", target_bir_lowering=False, debug=False,
                   num_devices=N_CORES)
    f32 = mybir.dt.float32
    f16 = mybir.dt.float16
    if internal_io:
        x = nc.dram_tensor("xbuf", [rows, N_FEAT], f32).ap()
        out = nc.dram_tensor("obuf", [rows, N_ACT], f16).ap()
        dummy = nc.dram_tensor("out", [1, 1], f32, kind="ExternalOutput").ap()
    else:
        x = nc.dram_tensor("x", [rows, N_FEAT], f32,
                           kind="ExternalInput").ap()
        out = nc.dram_tensor("out", [rows, N_ACT], f16,
                             kind="ExternalOutput").ap()

    # Chunk schedule: uniform r-row supertiles. (Tail-split variants
    # measured SLOWER on the load stream — 98.4 vs 95.7 us — and the
    # deferred-store schedule below removes the compute-tail motivation.)
    r_units = rows // P
    chunk_rs = [r] * (r_units // r)
    rest = r_units - sum(chunk_rs)
    if rest:
        chunk_rs.append(rest)
    if tail44 and chunk_rs[-1] == r and r % 2 == 0:
        chunk_rs[-1:] = [r // 2, r // 2]
    assert sum(chunk_rs) == r_units

    # Per chunk: partition p holds rr consecutive batch rows.
    def x_view(row0, rr):
        return x[row0:row0 + P * rr].rearrange("(p r) m -> p (r m)",
                                               p=P, r=rr)

    def o_view(row0, rr):
        return out[row0:row0 + P * rr].rearrange("(p r) m -> p (r m)",
                                                 p=P, r=rr)

    with TileContext(nc) as tc:
        with tc.tile_pool(name="xin", bufs=3) as xin_pool, \
             tc.tile_pool(name="y1", bufs=2) as y1_pool, \
             tc.tile_pool(name="y2", bufs=1) as y2_pool:
            if internal_io:
                # zero-fill the internal input region once so the bench
                # never squares NaN/Inf garbage, and feed the dummy output
                zt = xin_pool.tile([P, r * N_FEAT], f32, tag="xt")
                nc.gpsimd.memset(zt[:], 0.0)
                ztb = y2_pool.tile([P, r * N_ACT], f16, tag="y2z")
                nc.gpsimd.memset(ztb[:], 0.0)
                row0 = 0
                for rr in chunk_rs:
                    nc.sync.dma_start(out=x_view(row0, rr),
                                      in_=zt[:, :rr * N_FEAT])
                    row0 += P * rr
                nc.sync.dma_start(out=dummy, in_=zt[:1, :1])

            def body():
                row0 = 0
                if mode in ("inout", "inout_ser"):
                    st_engine = nc.gpsimd if mode == "inout" else nc.sync
                    for rr in chunk_rs:
                        xt = xin_pool.tile([P, rr * N_FEAT], f32, tag="xt")
                        nc.sync.dma_start(out=xt[:], in_=x_view(row0, rr))
                        row0 += P * rr
                    row0 = 0
                    for rr in chunk_rs:
                        st_engine.dma_start(out=o_view(row0, rr),
                                            in_=ztb[:, :rr * N_ACT])
                        row0 += P * rr
                    return
                if mode == "in2":
                    for t, rr in enumerate(chunk_rs):
                        xt = xin_pool.tile([P, rr * N_FEAT], f32, tag="xt")
                        eng = nc.sync if t % 2 == 0 else nc.scalar
                        eng.dma_start(out=xt[:], in_=x_view(row0, rr))
                        row0 += P * rr
                    return
                if mode == "out2":
                    for t, rr in enumerate(chunk_rs):
                        eng = nc.sync if t % 2 == 0 else nc.scalar
                        eng.dma_start(out=o_view(row0, rr),
                                      in_=ztb[:, :rr * N_ACT])
                        row0 += P * rr
                    return
                # (A single merged strided store DMA for chunks 0..5 was
                # tried and measured ~5 us SLOWER than per-chunk
                # contiguous stores — strided DRAM writes are penalized.)
                stores = []
                for t, rr in enumerate(chunk_rs):
                    if mode == "out_only":
                        nc.scalar.dma_start(out=o_view(row0, rr),
                                            in_=ztb[:, :rr * N_ACT])
                        row0 += P * rr
                        continue
                    xt = xin_pool.tile([P, rr * N_FEAT], f32, tag="xt")
                    nc.sync.dma_start(out=xt[:], in_=x_view(row0, rr))
                    if mode == "in_only":
                        row0 += P * rr
                        continue

                    # square in place (elementwise, same AP — safe). For
                    # the LAST chunk, split the square between ScalarE
                    # and VectorE: its chain runs after the final load
                    # with nothing left to hide it, so latency counts.
                    if split_last and t == len(chunk_rs) - 1 and rr > 1:
                        half = (rr // 2) * N_FEAT
                        nc.scalar.activation(
                            xt[:, :half], xt[:, :half],
                            mybir.ActivationFunctionType.Square)
                        nc.vector.tensor_mul(xt[:, half:], xt[:, half:],
                                             xt[:, half:])
                    else:
                        nc.scalar.activation(
                            xt[:], xt[:],
                            mybir.ActivationFunctionType.Square)

                    # pool over j: y1[p, 512rr], index = 512*row + 16*i + c
                    y1 = y1_pool.tile([P, rr * N_FEAT // 2], f32, tag="y1")
                    nc.vector.tensor_add(y1[:], xt[:, 0::2], xt[:, 1::2])

                    # pool over i: one dense add into y2 [p, rr*256]
                    # (y1 viewed [p, row, l, two, c]; y2 = even + odd i)
                    y1v = y1[:].rearrange("p (row l two c) -> p row l two c",
                                          row=rr, l=OUT_SIDE, two=2,
                                          c=OUT_SIDE)
                    y2t = y2_pool.tile([P, rr * N_ACT], f16,
                                       tag=f"y2_{t}", name=f"y2_{t}")
                    y2 = y2t[:]
                    stores.append((row0, rr, y2))
                    y2v = y2.rearrange("p (row l c) -> p row l c",
                                       row=rr, l=OUT_SIDE, c=OUT_SIDE)
                    nc.vector.tensor_add(y2v, y1v[:, :, :, 0, :],
                                         y1v[:, :, :, 1, :])
                    row0 += P * rr

                # All stores issued on the SYNC queue AFTER every load:
                # FIFO order serializes writes behind the read stream.
                # Measured: overlapped mixed read/write traffic runs at
                # 327 GB/s, strictly serialized streams at 351 GB/s —
                # read/write turnaround on HBM makes overlap a net loss.
                # By the time the loads drain, every chunk's compute but
                # the last is done, so the stores stream back-to-back.
                # store2: alternate stores between the sync queue and the
                # scalar queue. Scalar's stores trigger only after its
                # last square (engine program order), so they never mix
                # with the load stream; two queues hide per-DMA setup
                # gaps (writes measured 347 vs 313 GB/s).
                for i, (row0_s, rr, y2) in enumerate(stores):
                    eng = nc.scalar if (store2 and i % 2 == 1) else nc.sync
                    eng.dma_start(out=o_view(row0_s, rr), in_=y2)

            if repeat == 1:
                body()
            else:
                with tc.For_i(0, repeat, 1):
                    body()

    nc.compile()
    return nc


def kernel(**inputs) -> np.ndarray:
    x = np.ascontiguousarray(inputs["input_state"], dtype=np.float32)
    assert x.shape == (BATCH, N_FEAT), x.shape

    if "nc" not in _CACHE:
        _CACHE["nc"] = build_program()
    nc = _CACHE["nc"]

    shards = [x[i * ROWS_PER_CORE:(i + 1) * ROWS_PER_CORE]
              for i in range(N_CORES)]
    in_maps = [{"x": s} for s in shards]
    res = run_bass_kernel_spmd(nc, in_maps, list(range(N_CORES)))

    # gather + unshard: upcast the f16 device output, scatter the dense
    # 16-col blocks to R0[l] and materialize the always-zero columns
    compact = np.concatenate([np.asarray(res.results[i]["out"])
                              for i in range(N_CORES)], axis=0)
    compact = compact.astype(np.float32)
    full = np.zeros((BATCH, N_OUT), dtype=np.float32)
    for l in range(OUT_SIDE):
        full[:, R0[l]:R0[l] + OUT_SIDE] = \
            compact[:, l * OUT_SIDE:(l + 1) * OUT_SIDE]
    return full



# revision 32
# speedup vs baseline: 1.0788x; 1.0033x over previous
"""Trainium2 Bass kernel for nn_Custom_Final_Pooling_2D (segment_reduce).

Computes out = einsum("rn,bn->br", T, x*x) where T is the fixed binary
2x2-pooling selector built by the reference's build_pooling_matrix(32, 16):
  - T has shape [496, 1024]; only rows r0(l)+c are nonzero, where
    r0(l) = 31*l - l*(l+1)//2 + 15, for l, c in [0, 16).
  - Row r0(l)+c sums x[.., i*32+j]^2 over the 2x2 window
    i in {2l, 2l+1}, j in {2c, 2c+1}.

So the kernel is: square (ScalarE, in place), pairwise add along j
(VectorE, stride-2), pairwise add along i (VectorE) into the dense
[rows, 256] pool result stored as float16, then contiguous DMA stores
of that dense layout. The host upcasts to f32, scatters the 16 column
blocks to offsets r0(l) and materializes the 240 always-zero columns
while gathering. (Writing the 496- or 361-wide layouts on device moves
27-48% more bytes; strided partial-width stores measured ~1.9x slower
per byte than contiguous. f16 halves the store bytes again; the rel-err
cost is ~2e-4 against a 2e-2 budget.)

Schedule: the problem is HBM-bound (per-core: 32 MiB read + 4 MiB f16
write; read stream alone measures 96-101 us at ~350 GB/s). Measured on
HW: overlapped read+write traffic runs at ~327 GB/s aggregate while
strictly serialized pure streams run at ~351 GB/s — HBM read/write
turnaround makes overlap a net loss. So every output chunk is kept
live in SBUF and ALL stores issue after ALL loads (sync-queue FIFO
provides the ordering for free); by then all chunk computes except the
last have long finished and the writes stream at full bandwidth.

Data-parallel over 8 NeuronCores: batch dim sharded 65536 -> 8 x 8192.
"""

import numpy as np

import concourse.bacc as bacc
import concourse.mybir as mybir
from concourse.tile import TileContext
from concourse.bass_utils import run_bass_kernel_spmd

N_CORES = 8
BATCH = 65536
IMG = 32          # input image side
OUT_SIDE = 16     # pooled side
N_FEAT = IMG * IMG          # 1024
N_OUT = (2 * OUT_SIDE) * (2 * OUT_SIDE - 1) // 2  # 496
ROWS_PER_CORE = BATCH // N_CORES  # 8192

P = 128           # SBUF partitions
R = 8             # batch rows per partition per supertile
SUPER = P * R     # 1024 batch rows per supertile
N_TILES = ROWS_PER_CORE // SUPER  # 8

# Nonzero-row offsets of T: line l's 16 outputs live at columns
# r0(l) .. r0(l)+15 of the 496-wide output. Cols < 15 and >= 376 are
# always zero (as are the interior gaps); they stay at the memset value.
R0 = [31 * l - l * (l + 1) // 2 + 15 for l in range(OUT_SIDE)]

# The device writes the fully dense [rows, 256] pool output (line-major
# 16x16 blocks) — the exact nonzero values, contiguous, at full write
# bandwidth with 48% fewer bytes than the 496-wide layout; the host
# scatters the 16 column blocks to offsets R0[l] during the gather step.
N_ACT = OUT_SIDE * OUT_SIDE           # 256

# Output columns outside [OUT_LO, OUT_HI) are always zero.
OUT_LO = R0[0]                        # 15
OUT_HI = R0[OUT_SIDE - 1] + OUT_SIDE  # 376

_CACHE = {}


def build_program(rows: int = ROWS_PER_CORE, r: int = R, repeat: int = 1,
                  internal_io: bool = False, mode: str = "full",
                  split_last: bool = False, store2: bool = True,
                  tail44: bool = True):
    """Build the per-core Bass program: x [rows, 1024] -> out [rows, 496].

    repeat > 1 wraps the whole body in a hardware For_i loop that redoes
    the identical pass `repeat` times — used only for benchmarking (the
    slope over `repeat` isolates on-device time from host overhead).

    internal_io=True replaces the I/O tensors with internal DRAM buffers
    (plus a dummy [1,1] external output) so benchmark calls skip the
    256 MiB host<->device transfer entirely. The instruction stream is
    identical to the real program.

    mode: "full" (real kernel) | "in_only" | "out_only" | "inout"
    (loads + dep-free stores on separate queues, overlap allowed) |
    "inout_ser" (loads then stores on one queue, strictly serialized)
    — DMA stream probes for benchmarking.
    """
    nc = bacc.Bacc("TRN2", target_bir_lowering=False, debug=False,
                   num_devices=N_CORES)
    f32 = mybir.dt.float32
    f16 = mybir.dt.float16
    if internal_io:
        x = nc.dram_tensor("xbuf", [rows, N_FEAT], f32).ap()
        out = nc.dram_tensor("obuf", [rows, N_ACT], f16).ap()
        dummy = nc.dram_tensor("out", [1, 1], f32, kind="ExternalOutput").ap()
    else:
        x = nc.dram_tensor("x", [rows, N_FEAT], f32,
                           kind="ExternalInput").ap()
        out = nc.dram_tensor("out", [rows, N_ACT], f16,
                             kind="ExternalOutput").ap()

    # Chunk schedule: uniform r-row supertiles, with the final supertile
    # split in two (tail44): the last chunk's square->add->add chain runs
    # after the final load with nothing left to hide it, so halving the
    # last chunk halves that exposed tail. (Finer tails — 4,2,1,1 — only
    # fragment the load stream and measured slower.)
    r_units = rows // P
    chunk_rs = [r] * (r_units // r)
    rest = r_units - sum(chunk_rs)
    if rest:
        chunk_rs.append(rest)
    if tail44 and chunk_rs[-1] == r and r % 2 == 0:
        chunk_rs[-1:] = [r // 2, r // 2]
    assert sum(chunk_rs) == r_units

    # Per chunk: partition p holds rr consecutive batch rows.
    def x_view(row0, rr):
        return x[row0:row0 + P * rr].rearrange("(p r) m -> p (r m)",
                                               p=P, r=rr)

    def o_view(row0, rr):
        return out[row0:row0 + P * rr].rearrange("(p r) m -> p (r m)",
                                                 p=P, r=rr)

    with TileContext(nc) as tc:
        with tc.tile_pool(name="xin", bufs=3) as xin_pool, \
             tc.tile_pool(name="y1", bufs=2) as y1_pool, \
             tc.tile_pool(name="y2", bufs=1) as y2_pool:
            if internal_io:
                # zero-fill the internal input region once so the bench
                # never squares NaN/Inf garbage, and feed the dummy output
                zt = xin_pool.tile([P, r * N_FEAT], f32, tag="xt")
                nc.gpsimd.memset(zt[:], 0.0)
                ztb = y2_pool.tile([P, r * N_ACT], f16, tag="y2z")
                nc.gpsimd.memset(ztb[:], 0.0)
                row0 = 0
                for rr in chunk_rs:
                    nc.sync.dma_start(out=x_view(row0, rr),
                                      in_=zt[:, :rr * N_FEAT])
                    row0 += P * rr
                nc.sync.dma_start(out=dummy, in_=zt[:1, :1])

            def body():
                row0 = 0
                if mode in ("inout", "inout_ser"):
                    st_engine = nc.gpsimd if mode == "inout" else nc.sync
                    for rr in chunk_rs:
                        xt = xin_pool.tile([P, rr * N_FEAT], f32, tag="xt")
                        nc.sync.dma_start(out=xt[:], in_=x_view(row0, rr))
                        row0 += P * rr
                    row0 = 0
                    for rr in chunk_rs:
                        st_engine.dma_start(out=o_view(row0, rr),
                                            in_=ztb[:, :rr * N_ACT])
                        row0 += P * rr
                    return
                if mode == "in2":
                    for t, rr in enumerate(chunk_rs):
                        xt = xin_pool.tile([P, rr * N_FEAT], f32, tag="xt")
                        eng = nc.sync if t % 2 == 0 else nc.scalar
                        eng.dma_start(out=xt[:], in_=x_view(row0, rr))
                        row0 += P * rr
                    return
                if mode == "out2":
                    for t, rr in enumerate(chunk_rs):
                        eng = nc.sync if t % 2 == 0 else nc.scalar
                        eng.dma_start(out=o_view(row0, rr),
                                      in_=ztb[:, :rr * N_ACT])
                        row0 += P * rr
                    return
                # (A single merged strided store DMA for chunks 0..5 was
                # tried and measured ~5 us SLOWER than per-chunk
                # contiguous stores — strided DRAM writes are penalized.)
                stores = []
                for t, rr in enumerate(chunk_rs):
                    if mode == "out_only":
                        nc.scalar.dma_start(out=o_view(row0, rr),
                                            in_=ztb[:, :rr * N_ACT])
                        row0 += P * rr
                        continue
                    xt = xin_pool.tile([P, rr * N_FEAT], f32, tag="xt")
                    nc.sync.dma_start(out=xt[:], in_=x_view(row0, rr))
                    if mode == "in_only":
                        row0 += P * rr
                        continue

                    # square in place (elementwise, same AP — safe). For
                    # the LAST chunk, split the square between ScalarE
                    # and VectorE: its chain runs after the final load
                    # with nothing left to hide it, so latency counts.
                    if split_last and t == len(chunk_rs) - 1 and rr > 1:
                        half = (rr // 2) * N_FEAT
                        nc.scalar.activation(
                            xt[:, :half], xt[:, :half],
                            mybir.ActivationFunctionType.Square)
                        nc.vector.tensor_mul(xt[:, half:], xt[:, half:],
                                             xt[:, half:])
                    else:
                        nc.scalar.activation(
                            xt[:], xt[:],
                            mybir.ActivationFunctionType.Square)

                    # pool over j: y1[p, 512rr], index = 512*row + 16*i + c
                    y1 = y1_pool.tile([P, rr * N_FEAT // 2], f32, tag="y1")
                    nc.vector.tensor_add(y1[:], xt[:, 0::2], xt[:, 1::2])

                    # pool over i: one dense add into y2 [p, rr*256]
                    # (y1 viewed [p, row, l, two, c]; y2 = even + odd i)
                    y1v = y1[:].rearrange("p (row l two c) -> p row l two c",
                                          row=rr, l=OUT_SIDE, two=2,
                                          c=OUT_SIDE)
                    y2t = y2_pool.tile([P, rr * N_ACT], f16,
                                       tag=f"y2_{t}", name=f"y2_{t}")
                    y2 = y2t[:]
                    stores.append((row0, rr, y2))
                    y2v = y2.rearrange("p (row l c) -> p row l c",
                                       row=rr, l=OUT_SIDE, c=OUT_SIDE)
                    nc.vector.tensor_add(y2v, y1v[:, :, :, 0, :],
                                         y1v[:, :, :, 1, :])
                    row0 += P * rr

                # All stores issued on the SYNC queue AFTER every load:
                # FIFO order serializes writes behind the read stream.
                # Measured: overlapped mixed read/write traffic runs at
                # 327 GB/s, strictly serialized streams at 351 GB/s —
                # read/write turnaround on HBM makes overlap a net loss.
                # By the time the loads drain, every chunk's compute but
                # the last is done, so the stores stream back-to-back.
                # store2: alternate stores between the sync queue and the
                # scalar queue. Scalar's stores trigger only after its
                # last square (engine program order), so they never mix
                # with the load stream; two queues hide per-DMA setup
                # gaps (writes measured 347 vs 313 GB/s).
                for i, (row0_s, rr, y2) in enumerate(stores):
                    eng = nc.scalar if (store2 and i % 2 == 1) else nc.sync
                    eng.dma_start(out=o_view(row0_s, rr), in_=y2)

            if repeat == 1:
                body()
            else:
                with tc.For_i(0, repeat, 1):
                    body()

    nc.compile()
    return nc


def kernel(**inputs) -> np.ndarray:
    x = np.ascontiguousarray(inputs["input_state"], dtype=np.float32)
    assert x.shape == (BATCH, N_FEAT), x.shape

    if "nc" not in _CACHE:
        _CACHE["nc"] = build_program()
    nc = _CACHE["nc"]

    shards = [x[i * ROWS_PER_CORE:(i + 1) * ROWS_PER_CORE]
              for i in range(N_CORES)]
    in_maps = [{"x": s} for s in shards]
    res = run_bass_kernel_spmd(nc, in_maps, list(range(N_CORES)))

    # gather + unshard: upcast the f16 device output, scatter the dense
    # 16-col blocks to R0[l] and materialize the always-zero columns
    compact = np.concatenate([np.asarray(res.results[i]["out"])
                              for i in range(N_CORES)], axis=0)
    compact = compact.astype(np.float32)
    full = np.zeros((BATCH, N_OUT), dtype=np.float32)
    for l in range(OUT_SIDE):
        full[:, R0[l]:R0[l] + OUT_SIDE] = \
            compact[:, l * OUT_SIDE:(l + 1) * OUT_SIDE]
    return full



# revision 36
# speedup vs baseline: 1.0806x; 1.0017x over previous
"""Trainium2 Bass kernel for nn_Custom_Final_Pooling_2D (segment_reduce).

Computes out = einsum("rn,bn->br", T, x*x) where T is the fixed binary
2x2-pooling selector built by the reference's build_pooling_matrix(32, 16):
  - T has shape [496, 1024]; only rows r0(l)+c are nonzero, where
    r0(l) = 31*l - l*(l+1)//2 + 15, for l, c in [0, 16).
  - Row r0(l)+c sums x[.., i*32+j]^2 over the 2x2 window
    i in {2l, 2l+1}, j in {2c, 2c+1}.

So the kernel is: square (ScalarE, in place), pairwise add along j
(VectorE, stride-2), pairwise add along i (VectorE) into the dense
[rows, 256] pool result stored as float16, then contiguous DMA stores
of that dense layout. The host upcasts to f32, scatters the 16 column
blocks to offsets r0(l) and materializes the 240 always-zero columns
while gathering. (Writing the 496- or 361-wide layouts on device moves
27-48% more bytes; strided partial-width stores measured ~1.9x slower
per byte than contiguous. f16 halves the store bytes again; the rel-err
cost is ~2e-4 against a 2e-2 budget.)

Schedule: the problem is HBM-bound (per-core: 32 MiB read + 4 MiB f16
write; read stream alone measures 96-101 us at ~350 GB/s). Measured on
HW: overlapped read+write traffic runs at ~327 GB/s aggregate while
strictly serialized pure streams run at ~351 GB/s — HBM read/write
turnaround makes overlap a net loss. So every output chunk is kept
live in SBUF and ALL stores issue after ALL loads (sync-queue FIFO
provides the ordering for free); by then all chunk computes except the
last have long finished and the writes stream at full bandwidth.

Data-parallel over 8 NeuronCores: batch dim sharded 65536 -> 8 x 8192.
"""

import numpy as np

import concourse.bacc as bacc
import concourse.mybir as mybir
from concourse.tile import TileContext
from concourse.bass_utils import run_bass_kernel_spmd

N_CORES = 8
BATCH = 65536
IMG = 32          # input image side
OUT_SIDE = 16     # pooled side
N_FEAT = IMG * IMG          # 1024
N_OUT = (2 * OUT_SIDE) * (2 * OUT_SIDE - 1) // 2  # 496
ROWS_PER_CORE = BATCH // N_CORES  # 8192

P = 128           # SBUF partitions
R = 8             # batch rows per partition per supertile
SUPER = P * R     # 1024 batch rows per supertile
N_TILES = ROWS_PER_CORE // SUPER  # 8

# Nonzero-row offsets of T: line l's 16 outputs live at columns
# r0(l) .. r0(l)+15 of the 496-wide output. Cols < 15 and >= 376 are
# always zero (as are the interior gaps); they stay at the memset value.
R0 = [31 * l - l * (l + 1) // 2 + 15 for l in range(OUT_SIDE)]

# The device writes the fully dense [rows, 256] pool output (line-major
# 16x16 blocks) — the exact nonzero values, contiguous, at full write
# bandwidth with 48% fewer bytes than the 496-wide layout; the host
# scatters the 16 column blocks to offsets R0[l] during the gather step.
N_ACT = OUT_SIDE * OUT_SIDE           # 256

# Output columns outside [OUT_LO, OUT_HI) are always zero.
OUT_LO = R0[0]                        # 15
OUT_HI = R0[OUT_SIDE - 1] + OUT_SIDE  # 376

_CACHE = {}


def build_program(rows: int = ROWS_PER_CORE, r: int = R, repeat: int = 1,
                  internal_io: bool = False, mode: str = "full",
                  split_last: bool = False, store2: bool = True,
                  tail44: bool = True, xin_bufs: int = 3):
    """Build the per-core Bass program: x [rows, 1024] -> out [rows, 496].

    repeat > 1 wraps the whole body in a hardware For_i loop that redoes
    the identical pass `repeat` times — used only for benchmarking (the
    slope over `repeat` isolates on-device time from host overhead).

    internal_io=True replaces the I/O tensors with internal DRAM buffers
    (plus a dummy [1,1] external output) so benchmark calls skip the
    256 MiB host<->device transfer entirely. The instruction stream is
    identical to the real program.

    mode: "full" (real kernel) | "in_only" | "out_only" | "inout"
    (loads + dep-free stores on separate queues, overlap allowed) |
    "inout_ser" (loads then stores on one queue, strictly serialized)
    — DMA stream probes for benchmarking.
    """
    nc = bacc.Bacc("TRN2", target_bir_lowering=False, debug=False,
                   num_devices=N_CORES)
    f32 = mybir.dt.float32
    f16 = mybir.dt.float16
    if internal_io:
        x = nc.dram_tensor("xbuf", [rows, N_FEAT], f32).ap()
        out = nc.dram_tensor("obuf", [rows, N_ACT], f16).ap()
        dummy = nc.dram_tensor("out", [1, 1], f32, kind="ExternalOutput").ap()
    else:
        x = nc.dram_tensor("x", [rows, N_FEAT], f32,
                           kind="ExternalInput").ap()
        out = nc.dram_tensor("out", [rows, N_ACT], f16,
                             kind="ExternalOutput").ap()

    # Chunk schedule: uniform r-row supertiles, with the final supertile
    # split in two (tail44): the last chunk's square->add->add chain runs
    # after the final load with nothing left to hide it, so halving the
    # last chunk halves that exposed tail. (Finer tails — 4,2,1,1 — only
    # fragment the load stream and measured slower.)
    r_units = rows // P
    chunk_rs = [r] * (r_units // r)
    rest = r_units - sum(chunk_rs)
    if rest:
        chunk_rs.append(rest)
    if tail44 and chunk_rs[-1] == r and r % 2 == 0:
        chunk_rs[-1:] = [r // 2, r // 2]
    assert sum(chunk_rs) == r_units

    # Per chunk: partition p holds rr consecutive batch rows.
    def x_view(row0, rr):
        return x[row0:row0 + P * rr].rearrange("(p r) m -> p (r m)",
                                               p=P, r=rr)

    def o_view(row0, rr):
        return out[row0:row0 + P * rr].rearrange("(p r) m -> p (r m)",
                                                 p=P, r=rr)

    with TileContext(nc) as tc:
        with tc.tile_pool(name="xin", bufs=xin_bufs) as xin_pool, \
             tc.tile_pool(name="y1", bufs=2) as y1_pool, \
             tc.tile_pool(name="y2", bufs=1) as y2_pool:
            if internal_io:
                # zero-fill the internal input region once so the bench
                # never squares NaN/Inf garbage, and feed the dummy output
                zt = xin_pool.tile([P, r * N_FEAT], f32, tag="xt")
                nc.gpsimd.memset(zt[:], 0.0)
                ztb = y2_pool.tile([P, r * N_ACT], f16, tag="y2z")
                nc.gpsimd.memset(ztb[:], 0.0)
                row0 = 0
                for rr in chunk_rs:
                    nc.sync.dma_start(out=x_view(row0, rr),
                                      in_=zt[:, :rr * N_FEAT])
                    row0 += P * rr
                nc.sync.dma_start(out=dummy, in_=zt[:1, :1])

            def body():
                row0 = 0
                if mode in ("inout", "inout_ser"):
                    st_engine = nc.gpsimd if mode == "inout" else nc.sync
                    for rr in chunk_rs:
                        xt = xin_pool.tile([P, rr * N_FEAT], f32, tag="xt")
                        nc.sync.dma_start(out=xt[:], in_=x_view(row0, rr))
                        row0 += P * rr
                    row0 = 0
                    for rr in chunk_rs:
                        st_engine.dma_start(out=o_view(row0, rr),
                                            in_=ztb[:, :rr * N_ACT])
                        row0 += P * rr
                    return
                if mode == "in2":
                    for t, rr in enumerate(chunk_rs):
                        xt = xin_pool.tile([P, rr * N_FEAT], f32, tag="xt")
                        eng = nc.sync if t % 2 == 0 else nc.scalar
                        eng.dma_start(out=xt[:], in_=x_view(row0, rr))
                        row0 += P * rr
                    return
                if mode == "out2":
                    for t, rr in enumerate(chunk_rs):
                        eng = nc.sync if t % 2 == 0 else nc.scalar
                        eng.dma_start(out=o_view(row0, rr),
                                      in_=ztb[:, :rr * N_ACT])
                        row0 += P * rr
                    return
                # (A single merged strided store DMA for chunks 0..5 was
                # tried and measured ~5 us SLOWER than per-chunk
                # contiguous stores — strided DRAM writes are penalized.)
                stores = []
                for t, rr in enumerate(chunk_rs):
                    if mode == "out_only":
                        nc.scalar.dma_start(out=o_view(row0, rr),
                                            in_=ztb[:, :rr * N_ACT])
                        row0 += P * rr
                        continue
                    xt = xin_pool.tile([P, rr * N_FEAT], f32, tag="xt")
                    nc.sync.dma_start(out=xt[:], in_=x_view(row0, rr))
                    if mode == "in_only":
                        row0 += P * rr
                        continue
                    if mode == "in_sq":
                        nc.scalar.activation(
                            xt[:], xt[:],
                            mybir.ActivationFunctionType.Square)
                        row0 += P * rr
                        continue

                    # square in place (elementwise, same AP — safe). For
                    # the LAST chunk, split the square between ScalarE
                    # and VectorE: its chain runs after the final load
                    # with nothing left to hide it, so latency counts.
                    if split_last and t == len(chunk_rs) - 1 and rr > 1:
                        half = (rr // 2) * N_FEAT
                        nc.scalar.activation(
                            xt[:, :half], xt[:, :half],
                            mybir.ActivationFunctionType.Square)
                        nc.vector.tensor_mul(xt[:, half:], xt[:, half:],
                                             xt[:, half:])
                    else:
                        nc.scalar.activation(
                            xt[:], xt[:],
                            mybir.ActivationFunctionType.Square)

                    # pool over j: y1[p, 512rr], index = 512*row + 16*i + c
                    y1 = y1_pool.tile([P, rr * N_FEAT // 2], f32, tag="y1")
                    nc.vector.tensor_add(y1[:], xt[:, 0::2], xt[:, 1::2])

                    # pool over i: one dense add into y2 [p, rr*256]
                    # (y1 viewed [p, row, l, two, c]; y2 = even + odd i)
                    y1v = y1[:].rearrange("p (row l two c) -> p row l two c",
                                          row=rr, l=OUT_SIDE, two=2,
                                          c=OUT_SIDE)
                    y2t = y2_pool.tile([P, rr * N_ACT], f16,
                                       tag=f"y2_{t}", name=f"y2_{t}")
                    y2 = y2t[:]
                    stores.append((row0, rr, y2))
                    y2v = y2.rearrange("p (row l c) -> p row l c",
                                       row=rr, l=OUT_SIDE, c=OUT_SIDE)
                    nc.vector.tensor_add(y2v, y1v[:, :, :, 0, :],
                                         y1v[:, :, :, 1, :])
                    row0 += P * rr
                if mode == "in_cmp":
                    return  # loads + full compute, no stores

                # All stores issued on the SYNC queue AFTER every load:
                # FIFO order serializes writes behind the read stream.
                # Measured: overlapped mixed read/write traffic runs at
                # 327 GB/s, strictly serialized streams at 351 GB/s —
                # read/write turnaround on HBM makes overlap a net loss.
                # By the time the loads drain, every chunk's compute but
                # the last is done, so the stores stream back-to-back.
                # store2: alternate stores between the sync queue and the
                # scalar queue. Scalar's stores trigger only after its
                # last square (engine program order), so they never mix
                # with the load stream; two queues hide per-DMA setup
                # gaps (writes measured 347 vs 313 GB/s).
                for i, (row0_s, rr, y2) in enumerate(stores):
                    eng = nc.scalar if (store2 and i % 2 == 1) else nc.sync
                    eng.dma_start(out=o_view(row0_s, rr), in_=y2)

            if repeat == 1:
                body()
            else:
                with tc.For_i(0, repeat, 1):
                    body()

    nc.compile()
    return nc


def kernel(**inputs) -> np.ndarray:
    x = np.ascontiguousarray(inputs["input_state"], dtype=np.float32)
    assert x.shape == (BATCH, N_FEAT), x.shape

    if "nc" not in _CACHE:
        _CACHE["nc"] = build_program()
    nc = _CACHE["nc"]

    shards = [x[i * ROWS_PER_CORE:(i + 1) * ROWS_PER_CORE]
              for i in range(N_CORES)]
    in_maps = [{"x": s} for s in shards]
    res = run_bass_kernel_spmd(nc, in_maps, list(range(N_CORES)))

    # gather + unshard: upcast the f16 device output, scatter the dense
    # 16-col blocks to R0[l] and materialize the always-zero columns
    compact = np.concatenate([np.asarray(res.results[i]["out"])
                              for i in range(N_CORES)], axis=0)
    compact = compact.astype(np.float32)
    full = np.zeros((BATCH, N_OUT), dtype=np.float32)
    for l in range(OUT_SIDE):
        full[:, R0[l]:R0[l] + OUT_SIDE] = \
            compact[:, l * OUT_SIDE:(l + 1) * OUT_SIDE]
    return full

